# revision 7
# baseline (speedup 1.0000x reference)
"""Trainium2 Bass kernel for nn_ComplexAttention (B=8, C=512, H=W=32, HEADS=8).

Strategy
--------
Data-parallel over batch: one batch element per NeuronCore (8 cores), no
collectives.  Host-side algebraic fusion shrinks the per-core work:

  reference:  Q = R_q Wq Z,  K = R_k Wk Z,  V = R_v Wv Z   (complex, [C,T])
              S = Re(Q^H K)/sqrt(dh),  causal softmax -> A
              out = R_o Wo (V A^T)
  fused:      M = Wq^T diag(e^{i(phi_k-phi_q)}) Wk / sqrt(dh)   (host, f64)
              N = diag(e^{i phi_o}) Wo diag(e^{i phi_v}) Wv     (host, f64)
              Y = M Z            (channel-major [C,T])
              S = Re(Z^H Y)
              E = exp(causal(S)) (no max-subtraction: |S| < ~30)
              U = N Z            (token-major [T,C])
              out[t] = (E @ U)[t] / L[t],  L = row sums of E

Everything on-device is bf16 matmul / f32 PSUM.  End-to-end rel err
~7.9e-3 against the f64 oracle (budget 2e-2).

Schedule notes (from HW traces of the previous revision):
 - scores are computed TRANSPOSED (S^T[u,t] blocks, stationary = Y
   u-slice, streaming = Z) so the exp tiles are directly usable as the
   stationary operand of the attention-out matmuls -> no PE transposes,
   no DVE transpose copies.
 - attention out is TOKEN-major [t, c]; softmax normalization is a
   per-partition tensor_scalar during the PSUM->SBUF copy (free), with
   row sums L accumulated by N=1 matmuls against a ones vector that
   share the stationary weights of the out matmuls.  The host undoes
   the token-major layout during unsharding.
 - input DMA is spread across three queues (sync/scalar/gpsimd) in
   consumption order; one queue alone feeds ~200 GB/s which stalled
   the PE in the previous revision.
 - a memset tile + 6 dummy matmuls at the head of the PE queue keep
   the PE_HAM activity monitor busy during the DMA lead-in so the real
   matmul stream starts at 2.4 GHz instead of 1.2 GHz.
 - S^T block j is emitted one step ahead of out(j-1) so exp (scalar
   engine) always overlaps matmuls.
"""

import math

import numpy as np

import concourse.mybir as mybir
import concourse.tile as tile
from concourse import bacc
from concourse.bass_utils import run_bass_kernel_spmd

B, C, HH, WW = 8, 512, 32, 32
T = HH * WW          # 1024 tokens
DH = C // 8          # head dim (scale only)
P = 128
CT = C // P          # 4 channel tiles
TT = T // P          # 8 token tiles
NEG = -1.0e30
NDUMMY = 4

f32 = mybir.dt.float32
bf16 = mybir.dt.bfloat16


def _mm(nc, out, lhsT, rhs, start, stop):
    nc.tensor.matmul(out, lhsT, rhs, start=start, stop=stop)


_CACHE: dict = {}


def _get_program(has_imag: bool):
    key = has_imag
    if key not in _CACHE:
        _CACHE[key] = _build_program(has_imag)
    return _CACHE[key]


def _build_program(has_imag: bool):
    nc = bacc.Bacc("TRN2", target_bir_lowering=False, debug=False)

    zre_d = nc.dram_tensor("zre", [C, T], bf16, kind="ExternalInput").ap()
    zim_d = nc.dram_tensor("zim", [C, T], bf16, kind="ExternalInput").ap()
    mtre_d = nc.dram_tensor("mtre", [C, C], bf16, kind="ExternalInput").ap()
    ntre_d = nc.dram_tensor("ntre", [C, C], bf16, kind="ExternalInput").ap()
    if has_imag:
        mtim_d = nc.dram_tensor("mtim", [C, C], bf16, kind="ExternalInput").ap()
        mtimn_d = nc.dram_tensor("mtimn", [C, C], bf16, kind="ExternalInput").ap()
        ntim_d = nc.dram_tensor("ntim", [C, C], bf16, kind="ExternalInput").ap()
        ntimn_d = nc.dram_tensor("ntimn", [C, C], bf16, kind="ExternalInput").ap()
    trit_d = nc.dram_tensor("trit", [P, P], f32, kind="ExternalInput").ap()
    # token-major [T, C] outputs; the host transposes while unsharding
    outre_d = nc.dram_tensor("outre", [T, C], bf16, kind="ExternalOutput").ap()
    outim_d = nc.dram_tensor("outim", [T, C], bf16, kind="ExternalOutput").ap()

    with tile.TileContext(nc) as tc:
        with (
            tc.tile_pool(name="const", bufs=1) as cp,
            tc.tile_pool(name="work", bufs=4) as wp,
            tc.tile_pool(name="small", bufs=12) as sp,
            tc.tile_pool(name="psmm", bufs=6, space="PSUM") as pmm,
            tc.tile_pool(name="psl", bufs=2, space="PSUM") as psl,
        ):
            # -- constants (no DMA except the mask) + HAM warmup ------------
            dum = cp.tile([P, 512], bf16, tag="dum", name="dum")
            nc.gpsimd.memset(dum, 0.0)
            ones = cp.tile([P, 1], bf16, tag="ones", name="ones")
            nc.gpsimd.memset(ones, 1.0)
            trit = cp.tile([P, P], f32, tag="trit", name="trit")
            nc.gpsimd.dma_start(out=trit, in_=trit_d)

            pdum = pmm.tile([P, 512], f32, tag="mm", name="pdum")
            for _ in range(NDUMMY):
                nc.tensor.matmul(pdum, dum[:, :P], dum, start=True, stop=True)

            # -- persistent input tiles -------------------------------------
            mtre = [cp.tile([P, C], bf16, tag=f"mtre{c}", name=f"mtre{c}")
                    for c in range(CT)]
            ntre = [cp.tile([P, C], bf16, tag=f"ntre{c}", name=f"ntre{c}")
                    for c in range(CT)]
            zre_h = [[cp.tile([P, 512], bf16, tag=f"zre{c}_{h}",
                              name=f"zre{c}_{h}") for c in range(CT)]
                     for h in range(2)]
            zim_h = [[cp.tile([P, 512], bf16, tag=f"zim{c}_{h}",
                              name=f"zim{c}_{h}") for c in range(CT)]
                     for h in range(2)]

            def ld(eng, t, dram, r0, c0, w):
                eng.dma_start(out=t, in_=dram[r0:r0 + P, c0:c0 + w])

            # Loads in strict consumption order as (sync|scalar) pairs.
            # The DMA HW round-robins packets across ALL outstanding
            # transfers, so anything enqueued early delays the completion
            # of the critical first tiles -- keep zim/ntre strictly behind
            # the mtre/zre stream (measured: a third eager queue pushed
            # the first matmul from ~10.2us to 14.2us and re-throttled
            # the PE clock).
            pairs = [(mtre[c], mtre_d, c * P, 0, C) for c in range(CT)]
            zr = [(zre_h[h][c], zre_d, c * P, h * 512, 512)
                  for h in range(2) for c in range(CT)]
            zi = [(zim_h[h][c], zim_d, c * P, h * 512, 512)
                  for h in range(2) for c in range(CT)]
            nt = [(ntre[c], ntre_d, c * P, 0, C) for c in range(CT)]
            order = [pairs[0], zr[0], pairs[1], zr[1], pairs[2], zr[2],
                     pairs[3], zr[3], zr[4], zr[5], zr[6], zr[7]]
            order += nt
            if has_imag:
                mtim = [cp.tile([P, C], bf16, tag=f"mtim{c}") for c in range(CT)]
                mtimn = [cp.tile([P, C], bf16, tag=f"mtimn{c}") for c in range(CT)]
                ntim = [cp.tile([P, C], bf16, tag=f"ntim{c}") for c in range(CT)]
                ntimn = [cp.tile([P, C], bf16, tag=f"ntimn{c}") for c in range(CT)]
                order += [(mtimn[c], mtimn_d, c * P, 0, C) for c in range(CT)]
            order += zi
            if has_imag:
                order += [(mtim[c], mtim_d, c * P, 0, C) for c in range(CT)]
                order += [(ntim[c], ntim_d, c * P, 0, C) for c in range(CT)]
                order += [(ntimn[c], ntimn_d, c * P, 0, C) for c in range(CT)]
            for k, args in enumerate(order):
                ld((nc.sync, nc.scalar)[k % 2], *args)

            # -- Y = M Z (channel-major), U = N Z (token-major) -------------
            yre = [[cp.tile([P, 512], bf16, tag=f"yre{c}_{n}",
                            name=f"yre{c}_{n}") for n in range(2)]
                   for c in range(CT)]
            yim = [[cp.tile([P, 512], bf16, tag=f"yim{c}_{n}",
                            name=f"yim{c}_{n}") for n in range(2)]
                   for c in range(CT)]
            ure = [cp.tile([P, C], bf16, tag=f"ure{j}", name=f"ure{j}")
                   for j in range(TT)]
            uim = [cp.tile([P, C], bf16, tag=f"uim{j}", name=f"uim{j}")
                   for j in range(TT)]

            def emit_y(dst, terms):
                nterm = len(terms)
                for n in range(2):
                    pss = [pmm.tile([P, 512], f32, tag="mm", name="psmm")
                           for _ in range(CT)]
                    for t_i, (w, zh) in enumerate(terms):
                        for c in range(CT):
                            for m in range(CT):
                                _mm(nc, pss[m], w[c][:, m * P:(m + 1) * P],
                                    zh[n][c],
                                    start=(t_i == 0 and c == 0),
                                    stop=(t_i == nterm - 1 and c == CT - 1))
                    for m in range(CT):
                        nc.vector.tensor_copy(out=dst[m][n], in_=pss[m])

            def emit_u(dst, terms):
                for j in range(TT):
                    usl = slice((j % 4) * P, (j % 4 + 1) * P)
                    ps = pmm.tile([P, 512], f32, tag="mm", name="psmm")
                    nacc = len(terms) * CT
                    k = 0
                    for zh, w in terms:
                        for c in range(CT):
                            _mm(nc, ps, zh[j // 4][c][:, usl], w[c][:, :],
                                start=(k == 0), stop=(k == nacc - 1))
                            k += 1
                    nc.vector.tensor_copy(out=dst[j], in_=ps)

            if not has_imag:
                emit_y(yre, [(mtre, zre_h)])
                emit_u(ure, [(zre_h, ntre)])
                emit_y(yim, [(mtre, zim_h)])
                emit_u(uim, [(zim_h, ntre)])
            else:
                emit_y(yre, [(mtre, zre_h), (mtimn, zim_h)])
                emit_u(ure, [(zre_h, ntre), (zim_h, ntimn)])
                emit_y(yim, [(mtre, zim_h), (mtim, zre_h)])
                emit_u(uim, [(zim_h, ntre), (zre_h, ntim)])

            # -- transposed scores S^T[u,t] -> exp tiles E^T ----------------
            # block j covers u in [128j, 128j+128), t in [128j, 1024)
            sT = [cp.tile([P, T - j * P], bf16, tag=f"sT{j}", name=f"sT{j}")
                  for j in range(TT)]

            def emit_scores_block(j):
                base = j * P
                if j < 4:
                    chunks = [(base, 512), (512, 1024)]
                else:
                    chunks = [(base, 1024)]
                for a, b in chunks:
                    w = b - a
                    ps = pmm.tile([P, 512], f32, tag="mm", name="psmm")
                    k = 0
                    for zh, y in ((zre_h, yre), (zim_h, yim)):
                        for c in range(CT):
                            lhsT = y[c][j // 4][:, (j % 4) * P:(j % 4 + 1) * P]
                            rhs = zh[a // 512][c][:, a % 512:a % 512 + w]
                            _mm(nc, ps[:, :w], lhsT, rhs,
                                start=(k == 0), stop=(k == 2 * CT - 1))
                            k += 1
                    loc = a - base
                    if loc == 0:
                        # causal diagonal block: mask(+exp) via fr staging
                        fr = sp.tile([P, P], f32, tag="fr", name="fr")
                        nc.vector.tensor_add(out=fr, in0=ps[:, :P], in1=trit)
                        nc.scalar.activation(
                            out=sT[j][:, 0:P], in_=fr,
                            func=mybir.ActivationFunctionType.Exp)
                        if w > P:
                            nc.scalar.activation(
                                out=sT[j][:, P:w], in_=ps[:, P:w],
                                func=mybir.ActivationFunctionType.Exp)
                    else:
                        nc.scalar.activation(
                            out=sT[j][:, loc:loc + w], in_=ps[:, :w],
                            func=mybir.ActivationFunctionType.Exp)

            # -- attention out, token-major, normalization fused ------------
            def emit_out(i):
                ps_re = pmm.tile([P, 512], f32, tag="mm", name="psmm")
                ps_im = pmm.tile([P, 512], f32, tag="mm", name="psmm")
                ps_l = psl.tile([P, 1], f32, tag="l", name="psl")
                # pass A (re + row sums), then pass B (im): the reciprocal
                # and the re normalize/store overlap pass B's matmuls.
                for j in range(i + 1):
                    lhsT = sT[j][:, (i - j) * P:(i - j + 1) * P]
                    _mm(nc, ps_re, lhsT, ure[j], start=(j == 0), stop=(j == i))
                    _mm(nc, ps_l, lhsT, ones, start=(j == 0), stop=(j == i))
                rl = sp.tile([P, 1], f32, tag="rl", name="rl")
                nc.vector.reciprocal(out=rl, in_=ps_l)
                o_re = wp.tile([P, 512], bf16, tag="osb", name="osb")
                nc.vector.tensor_scalar_mul(o_re, ps_re, rl)
                # final block: split the im pass into halves so the last
                # normalize+store only exposes 64KB after the last matmul
                cols = ((0, 256), (256, 512)) if i == TT - 1 else ((0, 512),)
                tsl = slice(i * P, (i + 1) * P)
                nc.sync.dma_start(out=outre_d[tsl, :], in_=o_re)
                for c0, c1 in cols:
                    for j in range(i + 1):
                        lhsT = sT[j][:, (i - j) * P:(i - j + 1) * P]
                        _mm(nc, ps_im[:, c0:c1], lhsT, uim[j][:, c0:c1],
                            start=(j == 0), stop=(j == i))
                    o_im = wp.tile([P, 512], bf16, tag="osb", name="osb")
                    nc.vector.tensor_scalar_mul(
                        o_im[:, :c1 - c0], ps_im[:, c0:c1], rl)
                    nc.scalar.dma_start(out=outim_d[tsl, c0:c1],
                                        in_=o_im[:, :c1 - c0])

            emit_scores_block(0)
            for j in range(1, TT):
                emit_scores_block(j)
                emit_out(j - 1)
            emit_out(TT - 1)

    nc.compile()
    return nc


def _prep_weights(Wq, phi_q, Wk, phi_k, Wv, phi_v, Wo, phi_o):
    Wq, Wk, Wv, Wo = (np.asarray(w, np.float64) for w in (Wq, Wk, Wv, Wo))
    pq, pk, pv, po = (np.asarray(p, np.float64)
                      for p in (phi_q, phi_k, phi_v, phi_o))
    M = (Wq.T @ (np.exp(1j * (pk - pq))[:, None] * Wk)) / math.sqrt(DH)
    N = (np.exp(1j * po)[:, None] * Wo) @ (np.exp(1j * pv)[:, None] * Wv)
    has_imag = not (np.allclose(M.imag, 0.0) and np.allclose(N.imag, 0.0))
    return M, N, has_imag


def _consts(has_imag, M, N):
    import ml_dtypes
    snp = ml_dtypes.bfloat16
    consts = {
        "mtre": np.ascontiguousarray(M.real.T.astype(snp)),
        "ntre": np.ascontiguousarray(N.real.T.astype(snp)),
        # S^T diag-block causal mask: -inf where u > t (rows > cols)
        "trit": np.tril(np.full((P, P), NEG, np.float32), -1),
    }
    if has_imag:
        mtim = np.ascontiguousarray(M.imag.T.astype(snp))
        ntim = np.ascontiguousarray(N.imag.T.astype(snp))
        consts.update(mtim=mtim, mtimn=-mtim, ntim=ntim, ntimn=-ntim)
    return consts


def kernel(z_re, z_im, Wq, phi_q, Wk, phi_k, Wv, phi_v, Wo, phi_o):
    import ml_dtypes
    snp = ml_dtypes.bfloat16
    z_re = np.ascontiguousarray(np.asarray(z_re, np.float32).astype(snp))
    z_im = np.ascontiguousarray(np.asarray(z_im, np.float32).astype(snp))
    M, N, has_imag = _prep_weights(Wq, phi_q, Wk, phi_k, Wv, phi_v, Wo, phi_o)
    consts = _consts(has_imag, M, N)

    nc = _get_program(has_imag)
    in_maps = [
        dict(consts, zre=z_re[b].reshape(C, T), zim=z_im[b].reshape(C, T))
        for b in range(B)
    ]
    res = run_bass_kernel_spmd(nc, in_maps, list(range(B)))
    # device output is token-major [T, C]; transpose while unsharding
    out_re = np.stack([np.asarray(res.results[b]["outre"], np.float32)
                       .reshape(T, C).T.reshape(C, HH, WW) for b in range(B)])
    out_im = np.stack([np.asarray(res.results[b]["outim"], np.float32)
                       .reshape(T, C).T.reshape(C, HH, WW) for b in range(B)])
    return out_re, out_im


# revision 8
# speedup vs baseline: 1.1993x; 1.1993x over previous
"""Trainium2 Bass kernel for nn_ComplexAttention (B=8, C=512, H=W=32, HEADS=8).

Strategy
--------
Data-parallel over batch: one batch element per NeuronCore (8 cores), no
collectives.  Host-side algebraic fusion shrinks the per-core work:

  reference:  Q = R_q Wq Z,  K = R_k Wk Z,  V = R_v Wv Z   (complex, [C,T])
              S = Re(Q^H K)/sqrt(dh),  causal softmax -> A
              out = R_o Wo (V A^T)
  fused:      M = Wq^T diag(e^{i(phi_k-phi_q)}) Wk / sqrt(dh)   (host, f64)
              N = diag(e^{i phi_o}) Wo diag(e^{i phi_v}) Wv     (host, f64)
              Y = M Z            (channel-major [C,T])
              S = Re(Z^H Y)
              E = exp(causal(S)) (no max-subtraction: |S| < ~30)
              U = N Z            (token-major [T,C])
              out[t] = (E @ U)[t] / L[t],  L = row sums of E

Everything on-device is bf16 matmul / f32 PSUM.  End-to-end rel err
~7.9e-3 against the f64 oracle (budget 2e-2).

Schedule notes (from HW traces of the previous revision):
 - scores are computed TRANSPOSED (S^T[u,t] blocks, stationary = Y
   u-slice, streaming = Z) so the exp tiles are directly usable as the
   stationary operand of the attention-out matmuls -> no PE transposes,
   no DVE transpose copies.
 - attention out is TOKEN-major [t, c]; softmax normalization is a
   per-partition tensor_scalar during the PSUM->SBUF copy (free), with
   row sums L accumulated by N=1 matmuls against a ones vector that
   share the stationary weights of the out matmuls.  The host undoes
   the token-major layout during unsharding.
 - input DMA is spread across three queues (sync/scalar/gpsimd) in
   consumption order; one queue alone feeds ~200 GB/s which stalled
   the PE in the previous revision.
 - a memset tile + 6 dummy matmuls at the head of the PE queue keep
   the PE_HAM activity monitor busy during the DMA lead-in so the real
   matmul stream starts at 2.4 GHz instead of 1.2 GHz.
 - S^T block j is emitted one step ahead of out(j-1) so exp (scalar
   engine) always overlaps matmuls.
"""

import math

import numpy as np

import concourse.mybir as mybir
import concourse.tile as tile
from concourse import bacc
from concourse.bass_utils import run_bass_kernel_spmd

B, C, HH, WW = 8, 512, 32, 32
T = HH * WW          # 1024 tokens
DH = C // 8          # head dim (scale only)
P = 128
CT = C // P          # 4 channel tiles
TT = T // P          # 8 token tiles
NEG = -1.0e30
NDUMMY = 4

f32 = mybir.dt.float32
bf16 = mybir.dt.bfloat16


def _mm(nc, out, lhsT, rhs, start, stop):
    nc.tensor.matmul(out, lhsT, rhs, start=start, stop=stop)


_CACHE: dict = {}


def _get_program(has_imag: bool):
    key = has_imag
    if key not in _CACHE:
        _CACHE[key] = _build_program(has_imag)
    return _CACHE[key]


def _build_program(has_imag: bool):
    nc = bacc.Bacc("TRN2", target_bir_lowering=False, debug=False)

    zre_d = nc.dram_tensor("zre", [C, T], bf16, kind="ExternalInput").ap()
    zim_d = nc.dram_tensor("zim", [C, T], bf16, kind="ExternalInput").ap()
    mtre_d = nc.dram_tensor("mtre", [C, C], bf16, kind="ExternalInput").ap()
    ntre_d = nc.dram_tensor("ntre", [C, C], bf16, kind="ExternalInput").ap()
    if has_imag:
        mtim_d = nc.dram_tensor("mtim", [C, C], bf16, kind="ExternalInput").ap()
        mtimn_d = nc.dram_tensor("mtimn", [C, C], bf16, kind="ExternalInput").ap()
        ntim_d = nc.dram_tensor("ntim", [C, C], bf16, kind="ExternalInput").ap()
        ntimn_d = nc.dram_tensor("ntimn", [C, C], bf16, kind="ExternalInput").ap()
    trit_d = nc.dram_tensor("trit", [P, P], f32, kind="ExternalInput").ap()
    # token-major [T, C] outputs; the host transposes while unsharding
    outre_d = nc.dram_tensor("outre", [T, C], bf16, kind="ExternalOutput").ap()
    outim_d = nc.dram_tensor("outim", [T, C], bf16, kind="ExternalOutput").ap()

    with tile.TileContext(nc) as tc:
        with (
            tc.tile_pool(name="const", bufs=1) as cp,
            tc.tile_pool(name="work", bufs=4) as wp,
            tc.tile_pool(name="small", bufs=12) as sp,
            tc.tile_pool(name="psmm", bufs=6, space="PSUM") as pmm,
            tc.tile_pool(name="psl", bufs=2, space="PSUM") as psl,
        ):
            # -- constants (no DMA except the mask) + HAM warmup ------------
            dum = cp.tile([P, 512], bf16, tag="dum", name="dum")
            nc.gpsimd.memset(dum, 0.0)
            ones = cp.tile([P, 1], bf16, tag="ones", name="ones")
            nc.gpsimd.memset(ones, 1.0)
            trit = cp.tile([P, P], f32, tag="trit", name="trit")
            nc.gpsimd.dma_start(out=trit, in_=trit_d)

            pdum = pmm.tile([P, 512], f32, tag="mm", name="pdum")
            for _ in range(NDUMMY):
                nc.tensor.matmul(pdum, dum[:, :P], dum, start=True, stop=True)

            # -- persistent input tiles -------------------------------------
            mtre = [cp.tile([P, C], bf16, tag=f"mtre{c}", name=f"mtre{c}")
                    for c in range(CT)]
            ntre = [cp.tile([P, C], bf16, tag=f"ntre{c}", name=f"ntre{c}")
                    for c in range(CT)]
            zre_h = [[cp.tile([P, 512], bf16, tag=f"zre{c}_{h}",
                              name=f"zre{c}_{h}") for c in range(CT)]
                     for h in range(2)]
            zim_h = [[cp.tile([P, 512], bf16, tag=f"zim{c}_{h}",
                              name=f"zim{c}_{h}") for c in range(CT)]
                     for h in range(2)]

            def ld(eng, t, dram, r0, c0, w):
                eng.dma_start(out=t, in_=dram[r0:r0 + P, c0:c0 + w])

            # Loads in strict consumption order as (sync|scalar) pairs.
            # The DMA HW round-robins packets across ALL outstanding
            # transfers, so anything enqueued early delays the completion
            # of the critical first tiles -- keep zim/ntre strictly behind
            # the mtre/zre stream (measured: a third eager queue pushed
            # the first matmul from ~10.2us to 14.2us and re-throttled
            # the PE clock).
            pairs = [(mtre[c], mtre_d, c * P, 0, C) for c in range(CT)]
            zr = [(zre_h[h][c], zre_d, c * P, h * 512, 512)
                  for h in range(2) for c in range(CT)]
            zi = [(zim_h[h][c], zim_d, c * P, h * 512, 512)
                  for h in range(2) for c in range(CT)]
            nt = [(ntre[c], ntre_d, c * P, 0, C) for c in range(CT)]
            order = [pairs[0], zr[0], pairs[1], zr[1], pairs[2], zr[2],
                     pairs[3], zr[3], zr[4], zr[5], zr[6], zr[7]]
            order += nt
            if has_imag:
                mtim = [cp.tile([P, C], bf16, tag=f"mtim{c}") for c in range(CT)]
                mtimn = [cp.tile([P, C], bf16, tag=f"mtimn{c}") for c in range(CT)]
                ntim = [cp.tile([P, C], bf16, tag=f"ntim{c}") for c in range(CT)]
                ntimn = [cp.tile([P, C], bf16, tag=f"ntimn{c}") for c in range(CT)]
                order += [(mtimn[c], mtimn_d, c * P, 0, C) for c in range(CT)]
            order += zi
            if has_imag:
                order += [(mtim[c], mtim_d, c * P, 0, C) for c in range(CT)]
                order += [(ntim[c], ntim_d, c * P, 0, C) for c in range(CT)]
                order += [(ntimn[c], ntimn_d, c * P, 0, C) for c in range(CT)]
            for k, args in enumerate(order):
                ld((nc.sync, nc.scalar)[k % 2], *args)

            # -- Y = M Z (channel-major), U = N Z (token-major) -------------
            yre = [[cp.tile([P, 512], bf16, tag=f"yre{c}_{n}",
                            name=f"yre{c}_{n}") for n in range(2)]
                   for c in range(CT)]
            yim = [[cp.tile([P, 512], bf16, tag=f"yim{c}_{n}",
                            name=f"yim{c}_{n}") for n in range(2)]
                   for c in range(CT)]
            ure = [cp.tile([P, C], bf16, tag=f"ure{j}", name=f"ure{j}")
                   for j in range(TT)]
            uim = [cp.tile([P, C], bf16, tag=f"uim{j}", name=f"uim{j}")
                   for j in range(TT)]

            def emit_y(dst, terms):
                nterm = len(terms)
                for n in range(2):
                    pss = [pmm.tile([P, 512], f32, tag="mm", name="psmm")
                           for _ in range(CT)]
                    for t_i, (w, zh) in enumerate(terms):
                        for c in range(CT):
                            for m in range(CT):
                                _mm(nc, pss[m], w[c][:, m * P:(m + 1) * P],
                                    zh[n][c],
                                    start=(t_i == 0 and c == 0),
                                    stop=(t_i == nterm - 1 and c == CT - 1))
                    for m in range(CT):
                        nc.vector.tensor_copy(out=dst[m][n], in_=pss[m])

            def emit_u(dst, terms):
                for j in range(TT):
                    usl = slice((j % 4) * P, (j % 4 + 1) * P)
                    ps = pmm.tile([P, 512], f32, tag="mm", name="psmm")
                    nacc = len(terms) * CT
                    k = 0
                    for zh, w in terms:
                        for c in range(CT):
                            _mm(nc, ps, zh[j // 4][c][:, usl], w[c][:, :],
                                start=(k == 0), stop=(k == nacc - 1))
                            k += 1
                    nc.vector.tensor_copy(out=dst[j], in_=ps)

            if not has_imag:
                emit_y(yre, [(mtre, zre_h)])
                emit_u(ure, [(zre_h, ntre)])
                emit_y(yim, [(mtre, zim_h)])
                emit_u(uim, [(zim_h, ntre)])
            else:
                emit_y(yre, [(mtre, zre_h), (mtimn, zim_h)])
                emit_u(ure, [(zre_h, ntre), (zim_h, ntimn)])
                emit_y(yim, [(mtre, zim_h), (mtim, zre_h)])
                emit_u(uim, [(zim_h, ntre), (zre_h, ntim)])

            # -- transposed scores S^T[u,t] -> exp tiles E^T ----------------
            # block j covers u in [128j, 128j+128), t in [128j, 1024)
            sT = [cp.tile([P, T - j * P], bf16, tag=f"sT{j}", name=f"sT{j}")
                  for j in range(TT)]

            def emit_scores_block(j):
                base = j * P
                if j < 4:
                    chunks = [(base, 512), (512, 1024)]
                else:
                    chunks = [(base, 1024)]
                for a, b in chunks:
                    w = b - a
                    ps = pmm.tile([P, 512], f32, tag="mm", name="psmm")
                    k = 0
                    for zh, y in ((zre_h, yre), (zim_h, yim)):
                        for c in range(CT):
                            lhsT = y[c][j // 4][:, (j % 4) * P:(j % 4 + 1) * P]
                            rhs = zh[a // 512][c][:, a % 512:a % 512 + w]
                            _mm(nc, ps[:, :w], lhsT, rhs,
                                start=(k == 0), stop=(k == 2 * CT - 1))
                            k += 1
                    loc = a - base
                    if loc == 0:
                        # causal diagonal block: mask(+exp) via fr staging
                        fr = sp.tile([P, P], f32, tag="fr", name="fr")
                        nc.vector.tensor_add(out=fr, in0=ps[:, :P], in1=trit)
                        nc.scalar.activation(
                            out=sT[j][:, 0:P], in_=fr,
                            func=mybir.ActivationFunctionType.Exp)
                        if w > P:
                            nc.scalar.activation(
                                out=sT[j][:, P:w], in_=ps[:, P:w],
                                func=mybir.ActivationFunctionType.Exp)
                    else:
                        nc.scalar.activation(
                            out=sT[j][:, loc:loc + w], in_=ps[:, :w],
                            func=mybir.ActivationFunctionType.Exp)

            # -- attention out, token-major, normalization fused ------------
            def emit_out(i):
                ps_re = pmm.tile([P, 512], f32, tag="mm", name="psmm")
                ps_im = pmm.tile([P, 512], f32, tag="mm", name="psmm")
                ps_l = psl.tile([P, 1], f32, tag="l", name="psl")
                # pass A (re + row sums), then pass B (im): the reciprocal
                # and the re normalize/store overlap pass B's matmuls.
                for j in range(i + 1):
                    lhsT = sT[j][:, (i - j) * P:(i - j + 1) * P]
                    _mm(nc, ps_re, lhsT, ure[j], start=(j == 0), stop=(j == i))
                    _mm(nc, ps_l, lhsT, ones, start=(j == 0), stop=(j == i))
                rl = sp.tile([P, 1], f32, tag="rl", name="rl")
                nc.vector.reciprocal(out=rl, in_=ps_l)
                o_re = wp.tile([P, 512], bf16, tag="osb", name="osb")
                nc.vector.tensor_scalar_mul(o_re, ps_re, rl)
                # final block: split the im pass into halves so the last
                # normalize+store only exposes 64KB after the last matmul.
                # separate PSUM tiles per half: PSUM deps are bank-level,
                # so reusing one tile stalls half 2's matmuls on half 1's
                # normalize-copy (measured 0.7us).
                cols = ((0, 256), (256, 512)) if i == TT - 1 else ((0, 512),)
                tsl = slice(i * P, (i + 1) * P)
                nc.sync.dma_start(out=outre_d[tsl, :], in_=o_re)
                for ci, (c0, c1) in enumerate(cols):
                    ps = ps_im if ci == 0 else pmm.tile([P, 512], f32,
                                                        tag="mm", name="psmm")
                    for j in range(i + 1):
                        lhsT = sT[j][:, (i - j) * P:(i - j + 1) * P]
                        _mm(nc, ps[:, :c1 - c0], lhsT, uim[j][:, c0:c1],
                            start=(j == 0), stop=(j == i))
                    o_im = wp.tile([P, 512], bf16, tag="osb", name="osb")
                    nc.vector.tensor_scalar_mul(
                        o_im[:, :c1 - c0], ps[:, :c1 - c0], rl)
                    nc.scalar.dma_start(out=outim_d[tsl, c0:c1],
                                        in_=o_im[:, :c1 - c0])

            emit_scores_block(0)
            for j in range(1, TT):
                emit_scores_block(j)
                emit_out(j - 1)
            emit_out(TT - 1)

    nc.compile()
    return nc


def _prep_weights(Wq, phi_q, Wk, phi_k, Wv, phi_v, Wo, phi_o):
    Wq, Wk, Wv, Wo = (np.asarray(w, np.float64) for w in (Wq, Wk, Wv, Wo))
    pq, pk, pv, po = (np.asarray(p, np.float64)
                      for p in (phi_q, phi_k, phi_v, phi_o))
    M = (Wq.T @ (np.exp(1j * (pk - pq))[:, None] * Wk)) / math.sqrt(DH)
    N = (np.exp(1j * po)[:, None] * Wo) @ (np.exp(1j * pv)[:, None] * Wv)
    has_imag = not (np.allclose(M.imag, 0.0) and np.allclose(N.imag, 0.0))
    return M, N, has_imag


def _consts(has_imag, M, N):
    import ml_dtypes
    snp = ml_dtypes.bfloat16
    consts = {
        "mtre": np.ascontiguousarray(M.real.T.astype(snp)),
        "ntre": np.ascontiguousarray(N.real.T.astype(snp)),
        # S^T diag-block causal mask: -inf where u > t (rows > cols)
        "trit": np.tril(np.full((P, P), NEG, np.float32), -1),
    }
    if has_imag:
        mtim = np.ascontiguousarray(M.imag.T.astype(snp))
        ntim = np.ascontiguousarray(N.imag.T.astype(snp))
        consts.update(mtim=mtim, mtimn=-mtim, ntim=ntim, ntimn=-ntim)
    return consts


def kernel(z_re, z_im, Wq, phi_q, Wk, phi_k, Wv, phi_v, Wo, phi_o):
    import ml_dtypes
    snp = ml_dtypes.bfloat16
    z_re = np.ascontiguousarray(np.asarray(z_re, np.float32).astype(snp))
    z_im = np.ascontiguousarray(np.asarray(z_im, np.float32).astype(snp))
    M, N, has_imag = _prep_weights(Wq, phi_q, Wk, phi_k, Wv, phi_v, Wo, phi_o)
    consts = _consts(has_imag, M, N)

    nc = _get_program(has_imag)
    in_maps = [
        dict(consts, zre=z_re[b].reshape(C, T), zim=z_im[b].reshape(C, T))
        for b in range(B)
    ]
    res = run_bass_kernel_spmd(nc, in_maps, list(range(B)))
    # device output is token-major [T, C]; transpose while unsharding
    out_re = np.stack([np.asarray(res.results[b]["outre"], np.float32)
                       .reshape(T, C).T.reshape(C, HH, WW) for b in range(B)])
    out_im = np.stack([np.asarray(res.results[b]["outim"], np.float32)
                       .reshape(T, C).T.reshape(C, HH, WW) for b in range(B)])
    return out_re, out_im
